# revision 8
# baseline (speedup 1.0000x reference)
"""BessKGE DistMult shared-negative scoring on 8 NeuronCores.

Strategy: the reference's S=4 "shards" are a data dimension that degenerates
(the broadcast/all_to_all is identity on replicated data):
  - negative_embedding is shard 0's 1024 rows only,
  - negative_score (2048, 4096) is the (2048, 1024) score block tiled 4x
    along columns.
Real work: gather head/tail (2048 rows) + neg (1024 rows) from the
(250000, 256) entity table and rel (2048 rows) from the (1000, 256) relation
table, then G = head*rel, NS = G @ neg^T, P = rowsum(G*tail).

Distribution: entity/relation tables replicated per core; 2048 output rows
split 8 ways (256 rows per core). Each core gathers its rows on-device via
indirect DMA (the single SWDGE queue is the serial backbone of the kernel),
transposes operands on the PE to put E on partitions, runs the matmul with
K=256 accumulated over two K=128 halves, and writes its (256, 4096) output
slice. Host-side "unshard" is a pure row concatenation.

v4: gathers ordered so the first matmul's inputs land first and the output
transfers overlap the remaining gathers; identity loaded from DRAM to keep
gpsimd free for gathers; float32r single-pass matmuls; broadcast-source
output DMAs (each covers 2 of the 4 duplicate column blocks) alternating the
sync/scalar HW-DGE queues.
"""

import numpy as np

S = 4
PPP = 512
NEG = 1024
E = 256
N_ENT = 250000
N_REL = 1000
NCORES = 8
Q = S * PPP          # 2048 total query rows
R = Q // NCORES      # 256 rows per core
P = 128              # SBUF partitions
RT = R // P          # row tiles per core (2)
EK = E // P          # K halves (2)
NT = NEG // P        # negative gather tiles (8)
NCOL = 512           # matmul free-dim chunk (one PSUM bank)
NIDXCOL = NT + 3 * RT  # idx_all columns: 8 neg + 2 head + 2 rel + 2 tail

_CACHE = {}


def _build():
    import concourse.bacc as bacc
    import concourse.mybir as mybir
    import concourse.tile as tile
    from concourse import bass

    f32 = mybir.dt.float32
    f32r = mybir.dt.float32r
    i32 = mybir.dt.int32

    nc = bacc.Bacc(target_bir_lowering=False, debug=False, num_devices=NCORES,
                   enable_asserts=False)

    ent = nc.dram_tensor("ent", [N_ENT, E], f32, kind="ExternalInput").ap()
    relt = nc.dram_tensor("relt", [N_REL, E], f32, kind="ExternalInput").ap()
    # columns: 0..7 neg tiles, 8+t head, 10+t rel, 12+t tail
    idx_all = nc.dram_tensor("idx_all", [P, NIDXCOL], i32, kind="ExternalInput").ap()
    ident = nc.dram_tensor("ident", [P, P], f32, kind="ExternalInput").ap()
    neg_out = nc.dram_tensor("neg_out", [R, S * NEG], f32, kind="ExternalOutput").ap()
    pos_out = nc.dram_tensor("pos_out", [R], f32, kind="ExternalOutput").ap()

    with tile.TileContext(nc) as tc:
        with (
            tc.tile_pool(name="const", bufs=1) as const,
            tc.tile_pool(name="persist", bufs=1) as persist,
            tc.tile_pool(name="nrow_p", bufs=NT) as nrow_p,
            tc.tile_pool(name="sbuf", bufs=2) as sbuf,
            tc.tile_pool(name="ns_p", bufs=4) as ns_p,
            tc.tile_pool(name="tp_p", bufs=4, space="PSUM") as tp_p,
            tc.tile_pool(name="mm_p", bufs=2, space="PSUM") as mm_p,
        ):
            idx_t = const.tile([P, NIDXCOL], i32)
            nc.sync.dma_start(out=idx_t[:], in_=idx_all[:])
            identity = const.tile([P, P], f32)
            nc.scalar.dma_start(out=identity[:], in_=ident[:])

            def gather(dst_ap, table_ap, col):
                nc.gpsimd.indirect_dma_start(
                    out=dst_ap,
                    out_offset=None,
                    in_=table_ap,
                    in_offset=bass.IndirectOffsetOnAxis(
                        ap=idx_t[:, col:col + 1], axis=0),
                )

            nt_k = [persist.tile([P, NEG], f32r, tag=f"ntk{k}", name=f"ntk{k}")
                    for k in range(EK)]
            gt_k = [persist.tile([P, R], f32r, tag=f"gtk{k}", name=f"gtk{k}")
                    for k in range(EK)]

            def transpose_into(dst_ap, src_ap):
                pt = tp_p.tile([P, P], f32, tag="tp")
                nc.tensor.transpose(out=pt[:], in_=src_ap, identity=identity[:])
                nc.vector.tensor_copy(out=dst_ap, in_=pt[:])

            # --- gathers, ordered so ncol=0 matmul inputs land first ---
            n_tiles = [None] * NT
            for j in range(NT // 2):
                n_t = nrow_p.tile([P, E], f32, tag="nrow", name=f"nrow{j}")
                gather(n_t[:], ent[:], j)
                n_tiles[j] = n_t
            h_tiles, r_tiles, t_tiles = [], [], []
            for t in range(RT):
                h_t = sbuf.tile([P, E], f32, tag="h", name=f"h{t}")
                r_t = sbuf.tile([P, E], f32, tag="r", name=f"r{t}")
                gather(h_t[:], ent[:], NT + t)
                gather(r_t[:], relt[:], NT + RT + t)
                h_tiles.append(h_t)
                r_tiles.append(r_t)
            for j in range(NT // 2, NT):
                n_t = nrow_p.tile([P, E], f32, tag="nrow", name=f"nrow{j}")
                gather(n_t[:], ent[:], j)
                n_tiles[j] = n_t
            for t in range(RT):
                tt_t = sbuf.tile([P, E], f32, tag="t", name=f"t{t}")
                gather(tt_t[:], ent[:], NT + 2 * RT + t)
                t_tiles.append(tt_t)

            def mm_and_write(ncol, t, eng):
                mm = mm_p.tile([P, NCOL], f32, tag="mm")
                for k in range(EK):
                    nc.tensor.matmul(
                        out=mm[:],
                        lhsT=gt_k[k][:, t * P:(t + 1) * P],
                        rhs=nt_k[k][:, ncol * NCOL:(ncol + 1) * NCOL],
                        start=(k == 0),
                        stop=(k == EK - 1),
                    )
                ns = ns_p.tile([P, NCOL], f32, tag="ns")
                nc.vector.tensor_copy(out=ns[:], in_=mm[:])
                eng.dma_start(
                    out=neg_out[t * P:(t + 1) * P, :].rearrange(
                        "p (i n) -> p i n", i=S
                    )[:, :, ncol * NCOL:(ncol + 1) * NCOL],
                    in_=ns[:, None, :].to_broadcast([P, S, NCOL]),
                )

            # --- ncol=0 half: transposes, matmuls, writes (earliest data) ---
            for j in range(NT // 2):
                for k in range(EK):
                    transpose_into(nt_k[k][:, j * P:(j + 1) * P],
                                   n_tiles[j][:, k * P:(k + 1) * P])
            g_tiles = []
            for t in range(RT):
                g_t = sbuf.tile([P, E], f32, tag="g", name=f"g{t}")
                nc.vector.tensor_mul(out=g_t[:], in0=h_tiles[t][:], in1=r_tiles[t][:])
                g_tiles.append(g_t)
                for k in range(EK):
                    transpose_into(gt_k[k][:, t * P:(t + 1) * P],
                                   g_t[:, k * P:(k + 1) * P])
            mm_and_write(0, 0, nc.sync)
            mm_and_write(0, 1, nc.scalar)

            # --- ncol=1 half ---
            for j in range(NT // 2, NT):
                for k in range(EK):
                    transpose_into(nt_k[k][:, j * P:(j + 1) * P],
                                   n_tiles[j][:, k * P:(k + 1) * P])
            mm_and_write(1, 0, nc.gpsimd)
            mm_and_write(1, 1, nc.sync)

            # --- positives (short chain, gathers land last) ---
            for t in range(RT):
                scr = sbuf.tile([P, E], f32, tag="scr")
                pos_t = sbuf.tile([P, 1], f32, tag="pos")
                nc.vector.tensor_mul(out=scr[:], in0=g_tiles[t][:],
                                     in1=t_tiles[t][:])
                nc.vector.reduce_sum(out=pos_t[:], in_=scr[:],
                                     axis=mybir.AxisListType.X)
                nc.scalar.dma_start(out=pos_out[t * P:(t + 1) * P, None],
                                    in_=pos_t[:])

    nc.compile()
    return nc


def _get_nc():
    if "nc" not in _CACHE:
        _CACHE["nc"] = _build()
    return _CACHE["nc"]


def _make_idx_all(hflat, rflat, tflat, nflat, c):
    """Per-core (P, NIDXCOL) int32 index tile: col j<NT = neg tile j,
    then head, rel, tail row-tiles."""
    cols = []
    for j in range(NT):
        cols.append(nflat[j * P:(j + 1) * P])
    sl = slice(c * R, (c + 1) * R)
    for arr in (hflat, rflat, tflat):
        a = arr[sl]
        for t in range(RT):
            cols.append(a[t * P:(t + 1) * P])
    return np.ascontiguousarray(np.stack(cols, axis=1).astype(np.int32))


def kernel(head, relation, tail, negative, entity_embedding, relation_embedding):
    from concourse.bass_utils import run_bass_kernel_spmd

    head = np.asarray(head)
    relation = np.asarray(relation)
    tail = np.asarray(tail)
    negative = np.asarray(negative)
    ent = np.ascontiguousarray(np.asarray(entity_embedding, dtype=np.float32))
    relt = np.ascontiguousarray(np.asarray(relation_embedding, dtype=np.float32))

    hflat = head.reshape(-1).astype(np.int32)
    rflat = relation.reshape(-1).astype(np.int32)
    tflat = tail.reshape(-1).astype(np.int32)
    nflat = negative.reshape(S, -1)[0].astype(np.int32)
    ident = np.eye(P, dtype=np.float32)

    nc = _get_nc()
    in_maps = []
    for c in range(NCORES):
        in_maps.append({
            "ent": ent,
            "relt": relt,
            "ident": ident,
            "idx_all": _make_idx_all(hflat, rflat, tflat, nflat, c),
        })
    res = run_bass_kernel_spmd(nc, in_maps, core_ids=list(range(NCORES)))
    negative_score = np.concatenate(
        [res.results[c]["neg_out"] for c in range(NCORES)], axis=0
    )
    positive_score = np.concatenate(
        [res.results[c]["pos_out"] for c in range(NCORES)], axis=0
    )
    return positive_score, negative_score


# revision 9
# speedup vs baseline: 1.0514x; 1.0514x over previous
"""BessKGE DistMult shared-negative scoring on 8 NeuronCores.

Strategy: the reference's S=4 "shards" are a data dimension that degenerates
(the broadcast/all_to_all is identity on replicated data):
  - negative_embedding is shard 0's 1024 rows only,
  - negative_score (2048, 4096) is the (2048, 1024) score block tiled 4x
    along columns.
Real work: gather head/tail (2048 rows) + neg (1024 rows) from the
(250000, 256) entity table and rel (2048 rows) from the (1000, 256) relation
table, then G = head*rel, NS = G @ neg^T, P = rowsum(G*tail).

Distribution: entity/relation tables replicated per core; 2048 output rows
split 8 ways (256 rows per core). Each core gathers its rows on-device via
indirect DMA (the single SWDGE queue is the serial backbone of the kernel),
transposes operands on the PE to put E on partitions, runs the matmul with
K=256 accumulated over two K=128 halves, and writes its (256, 4096) output
slice. Host-side "unshard" is a pure row concatenation.

v4: gathers ordered so the first matmul's inputs land first and the output
transfers overlap the remaining gathers; identity loaded from DRAM to keep
gpsimd free for gathers; float32r single-pass matmuls; broadcast-source
output DMAs (each covers 2 of the 4 duplicate column blocks) alternating the
sync/scalar HW-DGE queues.
"""

import numpy as np

S = 4
PPP = 512
NEG = 1024
E = 256
N_ENT = 250000
N_REL = 1000
NCORES = 8
Q = S * PPP          # 2048 total query rows
R = Q // NCORES      # 256 rows per core
P = 128              # SBUF partitions
RT = R // P          # row tiles per core (2)
EK = E // P          # K halves (2)
NT = NEG // P        # negative gather tiles (8)
NCOL = 256           # matmul free-dim chunk
NIDXCOL = NT + 3 * RT  # idx_all columns: 8 neg + 2 head + 2 rel + 2 tail

_CACHE = {}


def _build():
    import concourse.bacc as bacc
    import concourse.mybir as mybir
    import concourse.tile as tile
    from concourse import bass

    f32 = mybir.dt.float32
    f32r = mybir.dt.float32r
    i32 = mybir.dt.int32

    nc = bacc.Bacc(target_bir_lowering=False, debug=False, num_devices=NCORES,
                   enable_asserts=False)

    ent = nc.dram_tensor("ent", [N_ENT, E], f32, kind="ExternalInput").ap()
    relt = nc.dram_tensor("relt", [N_REL, E], f32, kind="ExternalInput").ap()
    # columns: 0..7 neg tiles, 8+t head, 10+t rel, 12+t tail
    idx_all = nc.dram_tensor("idx_all", [P, NIDXCOL], i32, kind="ExternalInput").ap()
    ident = nc.dram_tensor("ident", [P, P], f32, kind="ExternalInput").ap()
    neg_out = nc.dram_tensor("neg_out", [R, S * NEG], f32, kind="ExternalOutput").ap()
    pos_out = nc.dram_tensor("pos_out", [R], f32, kind="ExternalOutput").ap()

    with tile.TileContext(nc) as tc:
        with (
            tc.tile_pool(name="const", bufs=1) as const,
            tc.tile_pool(name="persist", bufs=1) as persist,
            tc.tile_pool(name="nrow_p", bufs=NT) as nrow_p,
            tc.tile_pool(name="sbuf", bufs=2) as sbuf,
            tc.tile_pool(name="ns_p", bufs=4) as ns_p,
            tc.tile_pool(name="tp_p", bufs=4, space="PSUM") as tp_p,
            tc.tile_pool(name="mm_p", bufs=2, space="PSUM") as mm_p,
        ):
            idx_t = const.tile([P, NIDXCOL], i32)
            nc.sync.dma_start(out=idx_t[:], in_=idx_all[:])
            identity = const.tile([P, P], f32)
            nc.scalar.dma_start(out=identity[:], in_=ident[:])

            def gather(dst_ap, table_ap, col):
                nc.gpsimd.indirect_dma_start(
                    out=dst_ap,
                    out_offset=None,
                    in_=table_ap,
                    in_offset=bass.IndirectOffsetOnAxis(
                        ap=idx_t[:, col:col + 1], axis=0),
                )

            nt_k = [persist.tile([P, NEG], f32r, tag=f"ntk{k}", name=f"ntk{k}")
                    for k in range(EK)]
            gt_k = [persist.tile([P, R], f32r, tag=f"gtk{k}", name=f"gtk{k}")
                    for k in range(EK)]

            def transpose_into(dst_ap, src_ap):
                pt = tp_p.tile([P, P], f32, tag="tp")
                nc.tensor.transpose(out=pt[:], in_=src_ap, identity=identity[:])
                nc.vector.tensor_copy(out=dst_ap, in_=pt[:])

            # --- gathers, ordered so ncol=0 matmul inputs land first ---
            n_tiles = [None] * NT
            for j in range(NT // 2):
                n_t = nrow_p.tile([P, E], f32, tag="nrow", name=f"nrow{j}")
                gather(n_t[:], ent[:], j)
                n_tiles[j] = n_t
            h_tiles, r_tiles, t_tiles = [], [], []
            for t in range(RT):
                h_t = sbuf.tile([P, E], f32, tag="h", name=f"h{t}")
                r_t = sbuf.tile([P, E], f32, tag="r", name=f"r{t}")
                gather(h_t[:], ent[:], NT + t)
                gather(r_t[:], relt[:], NT + RT + t)
                h_tiles.append(h_t)
                r_tiles.append(r_t)
            for j in range(NT // 2, NT):
                n_t = nrow_p.tile([P, E], f32, tag="nrow", name=f"nrow{j}")
                gather(n_t[:], ent[:], j)
                n_tiles[j] = n_t
            for t in range(RT):
                tt_t = sbuf.tile([P, E], f32, tag="t", name=f"t{t}")
                gather(tt_t[:], ent[:], NT + 2 * RT + t)
                t_tiles.append(tt_t)

            def mm_and_write(ncol, t, eng):
                mm = mm_p.tile([P, NCOL], f32, tag="mm")
                for k in range(EK):
                    nc.tensor.matmul(
                        out=mm[:],
                        lhsT=gt_k[k][:, t * P:(t + 1) * P],
                        rhs=nt_k[k][:, ncol * NCOL:(ncol + 1) * NCOL],
                        start=(k == 0),
                        stop=(k == EK - 1),
                    )
                ns = ns_p.tile([P, NCOL], f32, tag="ns")
                nc.vector.tensor_copy(out=ns[:], in_=mm[:])
                eng.dma_start(
                    out=neg_out[t * P:(t + 1) * P, :].rearrange(
                        "p (i n) -> p i n", i=S
                    )[:, :, ncol * NCOL:(ncol + 1) * NCOL],
                    in_=ns[:, None, :].to_broadcast([P, S, NCOL]),
                )

            # --- chunked pipeline: per 2 neg tiles -> matmuls -> writes ---
            # chunk c consumes n_tiles[2c], n_tiles[2c+1]; G path before chunk 0
            for j in range(2):
                for k in range(EK):
                    transpose_into(nt_k[k][:, j * P:(j + 1) * P],
                                   n_tiles[j][:, k * P:(k + 1) * P])
            g_tiles = []
            for t in range(RT):
                g_t = sbuf.tile([P, E], f32, tag="g", name=f"g{t}")
                nc.vector.tensor_mul(out=g_t[:], in0=h_tiles[t][:], in1=r_tiles[t][:])
                g_tiles.append(g_t)
                for k in range(EK):
                    transpose_into(gt_k[k][:, t * P:(t + 1) * P],
                                   g_t[:, k * P:(k + 1) * P])
            engs = [nc.sync, nc.scalar, nc.sync, nc.scalar,
                    nc.gpsimd, nc.sync, nc.gpsimd, nc.scalar]
            w = 0
            for chunk in range(NT // 2):
                if chunk > 0:
                    for j in (2 * chunk, 2 * chunk + 1):
                        for k in range(EK):
                            transpose_into(nt_k[k][:, j * P:(j + 1) * P],
                                           n_tiles[j][:, k * P:(k + 1) * P])
                for t in range(RT):
                    mm_and_write(chunk, t, engs[w])
                    w += 1

            # --- positives (short chain, gathers land last) ---
            for t in range(RT):
                scr = sbuf.tile([P, E], f32, tag="scr")
                pos_t = sbuf.tile([P, 1], f32, tag="pos")
                nc.vector.tensor_mul(out=scr[:], in0=g_tiles[t][:],
                                     in1=t_tiles[t][:])
                nc.vector.reduce_sum(out=pos_t[:], in_=scr[:],
                                     axis=mybir.AxisListType.X)
                nc.scalar.dma_start(out=pos_out[t * P:(t + 1) * P, None],
                                    in_=pos_t[:])

    nc.compile()
    return nc


def _get_nc():
    if "nc" not in _CACHE:
        _CACHE["nc"] = _build()
    return _CACHE["nc"]


def _make_idx_all(hflat, rflat, tflat, nflat, c):
    """Per-core (P, NIDXCOL) int32 index tile: col j<NT = neg tile j,
    then head, rel, tail row-tiles."""
    cols = []
    for j in range(NT):
        cols.append(nflat[j * P:(j + 1) * P])
    sl = slice(c * R, (c + 1) * R)
    for arr in (hflat, rflat, tflat):
        a = arr[sl]
        for t in range(RT):
            cols.append(a[t * P:(t + 1) * P])
    return np.ascontiguousarray(np.stack(cols, axis=1).astype(np.int32))


def kernel(head, relation, tail, negative, entity_embedding, relation_embedding):
    from concourse.bass_utils import run_bass_kernel_spmd

    head = np.asarray(head)
    relation = np.asarray(relation)
    tail = np.asarray(tail)
    negative = np.asarray(negative)
    ent = np.ascontiguousarray(np.asarray(entity_embedding, dtype=np.float32))
    relt = np.ascontiguousarray(np.asarray(relation_embedding, dtype=np.float32))

    hflat = head.reshape(-1).astype(np.int32)
    rflat = relation.reshape(-1).astype(np.int32)
    tflat = tail.reshape(-1).astype(np.int32)
    nflat = negative.reshape(S, -1)[0].astype(np.int32)
    ident = np.eye(P, dtype=np.float32)

    nc = _get_nc()
    in_maps = []
    for c in range(NCORES):
        in_maps.append({
            "ent": ent,
            "relt": relt,
            "ident": ident,
            "idx_all": _make_idx_all(hflat, rflat, tflat, nflat, c),
        })
    res = run_bass_kernel_spmd(nc, in_maps, core_ids=list(range(NCORES)))
    negative_score = np.concatenate(
        [res.results[c]["neg_out"] for c in range(NCORES)], axis=0
    )
    positive_score = np.concatenate(
        [res.results[c]["pos_out"] for c in range(NCORES)], axis=0
    )
    return positive_score, negative_score
